# revision 61
# baseline (speedup 1.0000x reference)
"""Trainium2 Bass kernel for nn_CausalSelfAttention_16149077032974.

Full inputs in, full outputs out. Sharding: data-parallel over B (2 groups of
4 cores), tensor-parallel over heads within a group (4 heads/core). Each core
runs the whole per-head pipeline (QKVG projections, RoPE, QK-RMSNorm, causal
SDPA, output RMSNorm, silu gate, c_proj partial); the c_proj all-reduce is done
on the host while gathering (the partial sums are exact in f32).

Per-core kernel layout choices:
 - All inputs are host-permuted into partition-major DRAM layouts so every
   DMA reads multi-KB contiguous runs per partition line; the startup
   interleaves Wq and x[0:512] in 256KB k-pair pieces on one deep SP queue
   (a single deep queue sustains ~300GB/s; two shallow ones ~100 each) while
   all rope tables ride the ACT queue. ~60 dependency-free dummy matmuls on
   the identity warm the PE's HAM clock gate during the DMA wait.
 - Q/K are projected in (token, feature) layout where RoPE and RMSNorm are
   free-dim ops, then flipped per-head to (head_dim, token) with PE
   transposes. RMS rstd is computed on pre-RoPE values (rotation preserves
   per-token norms) so the normalization fuses into the PSUM evict.
 - Scores are built transposed, S^T[k, q] = K̂·Q̂^T, one 128-row k-tile pair
   per 2-bank PSUM tile; softmax needs no max subtraction (|S|·scale <=
   ~11.3 since q,k are RMS-normalized), so E = exp(scale·S^T) in one wide
   ACTIVATE per pair (diagonal pairs exp the full width; the below-diagonal
   garbage is finite and never read by PV).
 - The softmax denominator is never computed: the output RMSNorm divides by
   rms_d(y_un) directly (the eps·z² correction is <=0.6% of sumsq).
 - SDPA and c_proj run as ONE merged phase: the in-order PE queue is the
   scheduler, weaving score pairs (which gate on exp WAR with only 2 score
   buffers) between PV groups, output transposes and c_proj matmuls so no
   matmul waits at the queue head. ACT keeps a single table set (Exp) for
   the whole phase: 1/sqrt(m) is computed with an exponent-bits initial
   guess + Exp refine + one Newton step on DVE, never touching Sqrt.
 - q_gamma*k_gamma is folded into K̂ via pre-scaled rope tables; o_gamma is
   folded into Wo on the host.
"""

import numpy as np
import ml_dtypes
from contextlib import ExitStack

import concourse.bass as bass
import concourse.tile as tile
from concourse import bacc, mybir
from concourse import bass_utils
from concourse.bass import ts
from concourse.masks import make_identity, make_upper_triangular

BF16 = ml_dtypes.bfloat16
AL = mybir.AluOpType
F32 = mybir.dt.float32
BF = mybir.dt.bfloat16

B, T, H = 2, 2048, 2048
NH, HD = 16, 128
EPS = 1e-5
ROPE_BASE = 10000.0
NHL = 4          # heads per core
F = NHL * HD     # local feature width (512)
TT = T // 128    # 16 token tiles
CH = T // 512    # 4 query chunks
SCALE = 1.0 / float(np.sqrt(HD))

_CACHE = {}


def _build_nc():
    nc = bacc.Bacc("TRN2", target_bir_lowering=False, debug=False)

    # All inputs are pre-permuted on the host into partition-major layouts so
    # every DMA reads long contiguous runs per partition line.
    xT_d = nc.dram_tensor("xP", [128, TT, T], BF, kind="ExternalInput")
    wqT_d = nc.dram_tensor("wqP", [128, TT, F], BF, kind="ExternalInput")
    wkT_d = nc.dram_tensor("wkP", [128, TT, F], BF, kind="ExternalInput")
    wvT_d = nc.dram_tensor("wvP", [128, TT, F], BF, kind="ExternalInput")
    wgT_d = nc.dram_tensor("wgP", [128, TT, F], BF, kind="ExternalInput")
    woT_d = nc.dram_tensor("woP", [128, 4, H], BF, kind="ExternalInput")
    cos_d = nc.dram_tensor("cosb", [128, TT, HD], BF, kind="ExternalInput")
    sin_d = nc.dram_tensor("sinm", [128, TT, HD], BF, kind="ExternalInput")
    cosk_d = nc.dram_tensor("cosk", [128, TT, HD], BF, kind="ExternalInput")
    sink_d = nc.dram_tensor("sink", [128, TT, HD], BF, kind="ExternalInput")
    out_d = nc.dram_tensor("out", [T, H], BF, kind="ExternalOutput")

    with tile.TileContext(nc) as tc:
        with ExitStack() as outer:
            # ---- persistent pools (live across all phases) ----
            consts = outer.enter_context(tc.tile_pool(name="consts", bufs=1))
            qkt = outer.enter_context(tc.tile_pool(name="qkt", bufs=1))
            vpool = outer.enter_context(tc.tile_pool(name="vpool", bufs=1))
            gpool = outer.enter_context(tc.tile_pool(name="gpool", bufs=1))
            ypool = outer.enter_context(tc.tile_pool(name="ypool", bufs=1))
            spool = outer.enter_context(tc.tile_pool(name="spool", bufs=1))

            id128 = consts.tile([128, 128], BF, tag="id")
            make_identity(nc, id128[:, :])
            tri = consts.tile([128, 128], BF, tag="tri")
            make_upper_triangular(nc, tri[:, :], val=1.0, diag=True)
            eps_t = consts.tile([128, 1], F32, tag="eps")
            nc.vector.memset(eps_t[:, :], EPS)

            QT = [qkt.tile([128, T], BF, tag=f"qt{h}", name=f"QT{h}") for h in range(NHL)]
            KT = [qkt.tile([128, T], BF, tag=f"kt{h}", name=f"KT{h}") for h in range(NHL)]
            vaug = vpool.tile([128, TT, NHL, HD], BF, tag="vaug")
            gs = gpool.tile([128, TT, F], BF, tag="gs")       # silu(gate)
            yun = ypool.tile([128, TT, NHL, HD], BF, tag="yun")  # unnormalized y
            m_all = spool.tile([128, TT, NHL], F32, tag="mall")
            stok = spool.tile([128, TT, NHL], F32, tag="stok")

            # ================= phase A: projections =================
            with ExitStack() as pa:
                xpool = pa.enter_context(tc.tile_pool(name="xpool", bufs=1))
                wpool = pa.enter_context(tc.tile_pool(name="wpool", bufs=5))
                pcs = pa.enter_context(tc.tile_pool(name="pcs", bufs=1))
                pstage = pa.enter_context(tc.tile_pool(name="pstage", bufs=2))
                pqr = pa.enter_context(tc.tile_pool(name="pqr", bufs=4))
                pstat = pa.enter_context(tc.tile_pool(name="pstat", bufs=6))
                pqsq = pa.enter_context(tc.tile_pool(name="pqsq", bufs=1))
                psP = pa.enter_context(tc.tile_pool(name="psP", bufs=6, space="PSUM"))
                psTa = pa.enter_context(tc.tile_pool(name="psTa", bufs=2, space="PSUM"))

                xr = xT_d.ap()
                wqr = wqT_d.ap()

                # Interleaved startup on the SP queue in k-pair pieces (a
                # deep single queue sustains ~300GB/s; two shallow queues
                # only got ~100GB/s each): Wq pair then x[tokens 0:512] pair,
                # in warmup consumption order, then the bulk x slabs.
                wq_parts = []
                x0g = []
                for g in range(4):
                    wq_ = wpool.tile([128, 4, F], BF, tag="w", name=f"w_q_{g}")
                    xg = xpool.tile([128, 4, 512], BF, tag=f"x0g{g}", name=f"x0g{g}")
                    wq_parts.append(wq_)
                    x0g.append(xg)
                for g in range(4):
                    for half in range(2):
                        sl = slice(2 * half, 2 * half + 2)
                        kk = 4 * g + 2 * half
                        nc.sync.dma_start(
                            out=wq_parts[g][:, sl, :], in_=wqr[:, kk:kk + 2, :])
                        nc.sync.dma_start(
                            out=x0g[g][:, sl, :], in_=xr[:, kk:kk + 2, 0:512])
                # remaining x, in 512-token slabs (all k-groups each)
                xc = []
                for c in range(1, 4):
                    xt_ = xpool.tile([128, 16, 512], BF, tag=f"xc{c}", name=f"xc{c}")
                    nc.sync.dma_start(
                        out=xt_[:, :, :], in_=xr[:, :, 512 * c:512 * c + 512]
                    )
                    xc.append(xt_)
                # All rope tables go on the ACT queue (idle at startup): the
                # Q tables are needed from the first evicts; K's folded-gamma
                # tables are issued now too, because later the ACT queue is
                # busy with Q-phase evicts and the descriptor generation
                # would only run after those drain (measured +3us PE stall
                # at the Q->K seam otherwise).
                cr = cos_d.ap()
                sr = sin_d.ap()
                ckr = cosk_d.ap()
                skr = sink_d.ap()
                cos0 = pcs.tile([128, 4, HD], BF, tag="cos0")
                sin0 = pcs.tile([128, 4, HD], BF, tag="sin0")
                nc.scalar.dma_start(out=cos0[:, :, :], in_=cr[:, 0:4, :])
                nc.scalar.dma_start(out=sin0[:, :, :], in_=sr[:, 0:4, :])
                cos1 = pcs.tile([128, 12, HD], BF, tag="cos1")
                sin1 = pcs.tile([128, 12, HD], BF, tag="sin1")
                nc.scalar.dma_start(out=cos1[:, :, :], in_=cr[:, 4:16, :])
                nc.scalar.dma_start(out=sin1[:, :, :], in_=sr[:, 4:16, :])
                ck0 = pcs.tile([128, 4, HD], BF, tag="ck0")
                sk0 = pcs.tile([128, 4, HD], BF, tag="sk0")
                nc.scalar.dma_start(out=ck0[:, :, :], in_=ckr[:, 0:4, :])
                nc.scalar.dma_start(out=sk0[:, :, :], in_=skr[:, 0:4, :])
                ck1 = pcs.tile([128, 12, HD], BF, tag="ck1")
                sk1 = pcs.tile([128, 12, HD], BF, tag="sk1")
                nc.scalar.dma_start(out=ck1[:, :, :], in_=ckr[:, 4:16, :])
                nc.scalar.dma_start(out=sk1[:, :, :], in_=skr[:, 4:16, :])
                k_tables = (ck0, sk0, ck1, sk1)

                # Warm the PE's HAM clock gate during the startup DMA wait:
                # ~4us of dependency-free dummy matmuls flip the clock from
                # 1.2GHz to 2.4GHz before the first real projection arrives
                # (the array would otherwise run its first ~3.4us at half
                # rate). id128 is ready immediately; the PE is idle anyway.
                warm_ps = psP.tile([128, F], F32, tag="acc", name="hamwarm")
                for _ in range(60):
                    nc.tensor.matmul(
                        warm_ps[:, 0:128], id128[:, :], id128[:, :],
                        start=True, stop=True,
                    )

                def xap(k, t):
                    # x^T slice (128 h-part, 128 tokens) for k-tile k, t-tile t
                    if t < 4:
                        return x0g[k // 4][:, k % 4, ts(t, 128)]
                    return xc[t // 4 - 1][:, k, ts(t % 4, 128)]

                def cs_ap(t):
                    if t < 4:
                        return cos0[:, t, :], sin0[:, t, :]
                    return cos1[:, t - 4, :], sin1[:, t - 4, :]

                def load_w(w_d):
                    wr = w_d.ap()
                    parts = []
                    for q in range(4):
                        wq_ = wpool.tile([128, 4, F], BF, tag="w", name=f"w_{w_d.name}_{q}")
                        nc.sync.dma_start(out=wq_[:, :, :], in_=wr[:, 4 * q:4 * q + 4, :])
                        parts.append(wq_)
                    return parts

                def matmul_proj(acc, whalves, t):
                    for k in range(TT):
                        wt = whalves[k // 4]
                        nc.tensor.matmul(
                            acc[:, :], xap(k, t), wt[:, k % 4, :],
                            start=(k == 0), stop=(k == TT - 1),
                        )

                # -- Q then K: project + rmsnorm-prescale + rope --
                def qk_phase(targets, use_k_tables, whalves, is_q, warmup):
                    if use_k_tables:
                        ck0, sk0, ck1, sk1 = k_tables

                        def cs_k(t):
                            if t < 4:
                                return ck0[:, t, :], sk0[:, t, :]
                            return ck1[:, t - 4, :], sk1[:, t - 4, :]
                        get_cs = cs_k
                    else:
                        get_cs = cs_ap

                    pend = []

                    def flush(qr_t):
                        # PSUM->SBUF copies split 2 on ACT / 2 on DVE to
                        # balance the two engines' evict-chain load.
                        qr, t = qr_t
                        for hh in range(NHL):
                            tp = psTa.tile([128, 128], BF, tag="tp")
                            nc.tensor.transpose(tp[:, :], qr[:, hh, :], id128[:, :])
                            if hh < 2:
                                nc.scalar.copy(targets[hh][:, ts(t, 128)], tp[:, :])
                            else:
                                nc.vector.tensor_copy(targets[hh][:, ts(t, 128)], tp[:, :])

                    def evict(t, acc):
                        # mean(q^2)+eps per head -> rstd
                        msq = pstat.tile([128, NHL], F32, tag="msq")
                        scr = pqsq.tile([128, F], BF, tag="scr")
                        for hh in range(NHL):
                            nc.scalar.activation(
                                scr[:, ts(hh, 128)], acc[:, ts(hh, 128)],
                                mybir.ActivationFunctionType.Square,
                                accum_out=msq[:, hh:hh + 1],
                            )
                        # rstd = (msq/HD + eps)^-0.5 via exponent-bits guess
                        # + Exp refine + one Newton step: phase A then only
                        # ever touches the Exp table set (which also holds
                        # Square), so its single table load happens during
                        # the startup DMA wait instead of stalling the first
                        # evicts at ~31us.
                        sd = pstat.tile([128, NHL], F32, tag="sd")
                        nc.vector.tensor_scalar(
                            out=sd[:, :], in0=msq[:, :],
                            scalar1=1.0 / HD, scalar2=EPS,
                            op0=AL.mult, op1=AL.add,
                        )
                        rb = pstat.tile([128, NHL], F32, tag="rb")
                        nc.vector.tensor_copy(
                            rb[:, :], sd[:, :].bitcast(mybir.dt.int32))
                        nc.vector.tensor_scalar(
                            out=rb[:, :], in0=rb[:, :],
                            scalar1=2.0 ** -23, scalar2=-(127.0 + 0.043),
                            op0=AL.mult, op1=AL.add,
                        )
                        r0 = pstat.tile([128, NHL], F32, tag="r0")
                        nc.scalar.activation(
                            r0[:, :], rb[:, :],
                            mybir.ActivationFunctionType.Exp,
                            scale=-0.5 * float(np.log(2.0)),
                        )
                        ra = pstat.tile([128, NHL], F32, tag="ra")
                        nc.vector.tensor_mul(ra[:, :], r0[:, :], r0[:, :])
                        nc.vector.tensor_mul(ra[:, :], ra[:, :], sd[:, :])
                        nc.vector.tensor_scalar(
                            out=ra[:, :], in0=ra[:, :],
                            scalar1=-0.5, scalar2=1.5, op0=AL.mult, op1=AL.add,
                        )
                        rstd = pstat.tile([128, NHL], F32, tag="rstd")
                        nc.vector.tensor_mul(rstd[:, :], r0[:, :], ra[:, :])
                        qs = pstage.tile([128, NHL, HD], BF, tag="qs")
                        nc.vector.tensor_mul(
                            qs[:, :, :],
                            acc[:, :].rearrange("p (h d) -> p h d", h=NHL),
                            rstd[:, :, None].broadcast_to([128, NHL, HD]),
                        )
                        cost, sint = get_cs(t)
                        qc = pstage.tile([128, NHL, HD], BF, tag="qc")
                        nc.vector.tensor_mul(
                            qc[:, :, :], qs[:, :, :],
                            cost[:, None, :].broadcast_to([128, NHL, HD]),
                        )
                        rot = pstage.tile([128, NHL, HD], BF, tag="rot")
                        nc.vector.tensor_mul(
                            rot[:, :, 0:64], qs[:, :, 64:128],
                            sint[:, None, 0:64].broadcast_to([128, NHL, 64]),
                        )
                        nc.vector.tensor_mul(
                            rot[:, :, 64:128], qs[:, :, 0:64],
                            sint[:, None, 64:128].broadcast_to([128, NHL, 64]),
                        )
                        qr = pqr.tile([128, NHL, HD], BF, tag="qr")
                        nc.vector.tensor_add(qr[:, :, :], qc[:, :, :], rot[:, :, :])
                        pend.append((qr, t))
                        if len(pend) > 2:
                            flush(pend.pop(0))

                    if warmup:
                        # t=0..3 with 4 concurrently-open accumulators,
                        # consuming k-pairs in DMA arrival order
                        accs = [psP.tile([128, F], F32, tag="acc", name=f"warm{t}")
                                for t in range(4)]
                        for p in range(8):
                            for t in range(4):
                                for k in (2 * p, 2 * p + 1):
                                    nc.tensor.matmul(
                                        accs[t][:, :], xap(k, t),
                                        whalves[k // 4][:, k % 4, :],
                                        start=(k == 0), stop=(k == TT - 1),
                                    )
                        for t in range(4):
                            evict(t, accs[t])
                        t_range = range(4, TT)
                    else:
                        t_range = range(TT)

                    for t in t_range:
                        acc = psP.tile([128, F], F32, tag="acc")
                        matmul_proj(acc, whalves, t)
                        evict(t, acc)
                    while pend:
                        flush(pend.pop(0))

                qk_phase(QT, False, wq_parts, True, True)
                qk_phase(KT, True, load_w(wkT_d), False, False)

                # -- V --
                whalves = load_w(wvT_d)
                for t in range(TT):
                    acc = psP.tile([128, F], F32, tag="acc")
                    matmul_proj(acc, whalves, t)
                    nc.scalar.copy(
                        vaug[:, t, :, :],
                        acc[:, :].rearrange("p (h d) -> p h d", h=NHL),
                    )

                # -- G (silu fused into the evict) --
                whalves = load_w(wgT_d)
                for t in range(TT):
                    acc = psP.tile([128, F], F32, tag="acc")
                    matmul_proj(acc, whalves, t)
                    nc.scalar.activation(
                        gs[:, t, :], acc[:, :], mybir.ActivationFunctionType.Silu
                    )
            # woT is only consumed in phase C; allocating + loading it here
            # (after phase A's pools are released) keeps it out of phase A's
            # SBUF high-water mark and the 2MB transfer hides under phase B.
            wopool = outer.enter_context(tc.tile_pool(name="wopool", bufs=1))
            woT = wopool.tile([128, 4, H], BF, tag="woT")
            nc.sync.dma_start(out=woT[:, :, :], in_=woT_d.ap())

            # ========== phase BC: SDPA with c_proj interleaved ==========
            # Phase B alone is dependency-bound (matmul->exp->PV with 2-3
            # PSUM score buffers leaves the PE idle at unit boundaries) and
            # phase C alone is pure PE work; interleaving C's tiles into the
            # B unit stream fills both engines' stalls.
            with ExitStack() as pb:
                pE = pb.enter_context(tc.tile_pool(name="pE", bufs=21))
                pys = pb.enter_context(tc.tile_pool(name="pys", bufs=4))
                pyscr = pb.enter_context(tc.tile_pool(name="pyscr", bufs=2))
                prs = pb.enter_context(tc.tile_pool(name="prs", bufs=2))
                pyg = pb.enter_context(tc.tile_pool(name="pyg", bufs=10))
                pygT = pb.enter_context(tc.tile_pool(name="pygT", bufs=24))
                pout = pb.enter_context(tc.tile_pool(name="pout", bufs=6))
                psO = pb.enter_context(tc.tile_pool(name="psO", bufs=1, space="PSUM"))
                psTc = pb.enter_context(tc.tile_pool(name="psTc", bufs=1, space="PSUM"))
                pbs = pb.enter_context(ExitStack())
                psS = pbs.enter_context(tc.tile_pool(name="psS", bufs=2, space="PSUM"))
                psY = pbs.enter_context(tc.tile_pool(name="psY", bufs=2, space="PSUM"))
                cpools = {"O": psO, "T": psTc}

                def emit_pair(c, h, elist, kt0):
                    # one S^T pair: two k-tile matmuls + one wide exp.
                    # Diagonal matmuls only compute the causal columns; the
                    # exp still covers the full pair in one ACTIVATE (the
                    # below-diagonal region holds stale PSUM garbage whose
                    # exp is finite and never read by PV).
                    e2 = pE.tile([128, 2, 512], BF, tag="e")
                    s2 = psS.tile([128, 2, 512], F32, tag="s")
                    diag = kt0 >= 4 * c
                    for j in range(2):
                        kt = kt0 + j
                        lo = 0 if kt < 4 * c else 128 * (kt - 4 * c)
                        nc.tensor.matmul(
                            s2[:, j, lo:512], KT[h][:, ts(kt, 128)],
                            QT[h][:, 512 * c + lo:512 * c + 512],
                            start=True, stop=True,
                        )
                    nc.scalar.activation(
                        e2[:, :, :], s2[:, :, :],
                        mybir.ActivationFunctionType.Exp, scale=SCALE,
                    )
                    if diag:
                        for j in range(2):
                            lo = 128 * (kt0 + j - 4 * c)
                            nc.vector.tensor_mul(
                                e2[:, j, lo:lo + 128], e2[:, j, lo:lo + 128],
                                tri[:, :],
                            )
                    elist[kt0 // 2] = e2

                def pv_group(c, h, elist, p):
                    # PV for one psY group (two query tiles); evict is one
                    # CAST, stats are fused square+reduce from SBUF after
                    y_ps = psY.tile([128, 2, HD], F32, tag="y")
                    for qp in range(2):
                        qt = 2 * p + qp
                        t = 4 * c + qt
                        for kt in range(t + 1):
                            nc.tensor.matmul(
                                y_ps[:, qp, :],
                                elist[kt // 2][:, kt % 2, ts(qt, 128)],
                                vaug[:, kt, h, :],
                                start=(kt == 0), stop=(kt == t),
                            )
                    tpair = 4 * c + 2 * p
                    nc.vector.tensor_copy(
                        yun[:, tpair:tpair + 2, h, :], y_ps[:, :, :]
                    )
                    scr = pyscr.tile([128, 2, HD], BF, tag="ysq")
                    nc.vector.tensor_mul(
                        scr[:, :, :], yun[:, tpair:tpair + 2, h, :],
                        yun[:, tpair:tpair + 2, h, :],
                    )
                    ss = pys.tile([128, 2], F32, tag="ss")
                    nc.vector.tensor_reduce(
                        out=ss[:, :], in_=scr[:, :, :],
                        axis=mybir.AxisListType.X, op=mybir.AluOpType.add,
                    )
                    nc.vector.tensor_scalar(
                        out=m_all[:, tpair:tpair + 2, h], in0=ss[:, :],
                        scalar1=1.0 / HD, scalar2=None, op0=AL.mult,
                    )

                def stok_part(c, h_lo, h_hi):
                    # stok = m^-0.5 without touching any new ACT table set:
                    # exponent-bits initial guess (Mitchell), refined through
                    # the already-loaded Exp table, then one Newton step on
                    # DVE (max rel err ~3e-4).
                    msl = m_all[:, 4 * c:4 * c + 4, h_lo:h_hi]
                    shp = [128, 4, h_hi - h_lo]
                    bf_ = prs.tile(shp, F32, tag="rsb")
                    nc.vector.tensor_copy(
                        bf_[:, :, :], msl.bitcast(mybir.dt.int32))
                    l2 = prs.tile(shp, F32, tag="rsl")
                    nc.vector.tensor_scalar(
                        out=l2[:, :, :], in0=bf_[:, :, :],
                        scalar1=2.0 ** -23, scalar2=-(127.0 + 0.043),
                        op0=AL.mult, op1=AL.add,
                    )
                    r0 = prs.tile(shp, F32, tag="rs0")
                    nc.scalar.activation(
                        r0[:, :, :], l2[:, :, :],
                        mybir.ActivationFunctionType.Exp,
                        scale=-0.5 * float(np.log(2.0)),
                    )
                    aa = prs.tile(shp, F32, tag="rsa")
                    nc.vector.tensor_mul(aa[:, :, :], r0[:, :, :], r0[:, :, :])
                    nc.vector.tensor_mul(aa[:, :, :], aa[:, :, :], msl)
                    cc = prs.tile(shp, F32, tag="rsc")
                    nc.vector.tensor_scalar(
                        out=cc[:, :, :], in0=aa[:, :, :],
                        scalar1=-0.5, scalar2=1.5, op0=AL.mult, op1=AL.add,
                    )
                    nc.vector.tensor_mul(
                        stok[:, 4 * c:4 * c + 4, h_lo:h_hi],
                        r0[:, :, :], cc[:, :, :])

                tilesT = {}

                def c_trans(t, hh):
                    # one head's normalize+gate+transpose (all-DVE except the
                    # PSUM evict split); tiny PE item, so psTc gets by with a
                    # single bank when these are woven between heavier items.
                    if t not in tilesT:
                        tilesT[t] = [None] * NHL
                    t1 = pyg.tile([128, HD], BF, tag="t1")
                    nc.vector.tensor_mul(
                        t1[:, :], yun[:, t, hh, :], gs[:, t, ts(hh, 128)]
                    )
                    yg = pyg.tile([128, HD], BF, tag="yg", name=f"yg_{t}_{hh}")
                    nc.vector.tensor_scalar_mul(
                        yg[:, :], t1[:, :], stok[:, t, hh:hh + 1]
                    )
                    tp = cpools["T"].tile([128, 128], BF, tag="tp")
                    nc.tensor.transpose(tp[:, :], yg[:, :], id128[:, :])
                    yt = pygT.tile([128, 128], BF, tag="yt")
                    if hh % 2 == 0:
                        nc.scalar.copy(yt[:, :], tp[:, :])
                    else:
                        nc.vector.tensor_copy(yt[:, :], tp[:, :])
                    tilesT[t][hh] = yt

                def c_proj(t, n):
                    o_ps = cpools["O"].tile([128, 512], F32, tag="o")
                    for f in range(4):
                        nc.tensor.matmul(
                            o_ps[:, :], tilesT[t][f][:, :], woT[:, f, ts(n, 512)],
                            start=(f == 0), stop=(f == 3),
                        )
                    o_sb = pout.tile([128, 512], BF, tag="osb")
                    if n % 2 == 0:
                        nc.vector.tensor_copy(o_sb[:, :], o_ps[:, :])
                    else:
                        nc.scalar.copy(o_sb[:, :], o_ps[:, :])
                    nc.sync.dma_start(
                        out=out_d.ap()[ts(t, 128), ts(n, 512)], in_=o_sb[:, :]
                    )

                # The in-order PE queue is the scheduler: weave the score
                # pairs (which gate on exp WAR with only 2 score buffers)
                # between the PV / transpose / c_proj items so no matmul ever
                # sits at the queue head waiting for the ACT engine.
                units = [(c, h) for c in range(CH) for h in range(NHL)]
                prev = None
                cwork = []   # pending C work items (thunks), in order
                for (c, h) in units:
                    elist = [None] * (2 * c + 2)
                    kt0_order = [4 * c, 4 * c + 2] + list(range(0, 4 * c, 2))
                    sitems = [
                        (lambda kt0=kt0: emit_pair(c, h, elist, kt0))
                        for kt0 in kt0_order
                    ]
                    oitems = []
                    if prev is not None:
                        pc_, ph_, pel = prev
                        oitems.append(lambda: pv_group(pc_, ph_, pel, 0))
                        oitems.append(lambda: pv_group(pc_, ph_, pel, 1))
                        if ph_ == NHL - 1:
                            oitems.append(lambda: stok_part(pc_, 0, NHL))
                            for j in range(4):
                                tt = 4 * pc_ + j
                                for hh in range(NHL):
                                    cwork.append(
                                        (lambda tt=tt, hh=hh: c_trans(tt, hh)))
                                for n in range(4):
                                    cwork.append(
                                        (lambda tt=tt, n=n: c_proj(tt, n)))
                        if (pc_, ph_) == (CH - 1, NHL - 2):
                            # last chunk, second-to-last head done: compute
                            # its stok + transposes for heads 0..2 NOW so the
                            # post-loop tail only owes head 3's serial chain
                            oitems.append(lambda: stok_part(CH - 1, 0, NHL - 1))
                            for j in range(4):
                                tt = 4 * (CH - 1) + j
                                for hh in range(NHL - 1):
                                    oitems.append(
                                        (lambda tt=tt, hh=hh: c_trans(tt, hh)))
                    # take this unit's share of pending C work (8 items per
                    # unit drains a chunk's 32 items over its 4 units)
                    take, cwork = cwork[:8], cwork[8:]
                    oitems += take
                    # weave: two score pairs up front (prime the exp
                    # pipeline), then alternate
                    emit = []
                    si, oi = 0, 0
                    while si < len(sitems) or oi < len(oitems):
                        if si < 2 and si < len(sitems):
                            emit.append(sitems[si]); si += 1
                            continue
                        if oi < len(oitems):
                            emit.append(oitems[oi]); oi += 1
                        if si < len(sitems):
                            emit.append(sitems[si]); si += 1
                    for fn in emit:
                        fn()
                    prev = (c, h, elist)
                # tail: last unit's PV + stok, then release the score/PV
                # PSUM pools so the final chunk's c_proj runs with proper
                # double buffering instead of single-bank serialization
                pc_, ph_, pel = prev
                pv_group(pc_, ph_, pel, 0)
                pv_group(pc_, ph_, pel, 1)
                stok_part(pc_, NHL - 1, NHL)
                pbs.close()
                cpools["O"] = pb.enter_context(
                    tc.tile_pool(name="ptO", bufs=3, space="PSUM"))
                cpools["T"] = pb.enter_context(
                    tc.tile_pool(name="ptT", bufs=2, space="PSUM"))
                for j in range(4):
                    tt = 12 + j
                    cwork.append((lambda tt=tt: c_trans(tt, NHL - 1)))
                    for n in range(4):
                        cwork.append((lambda tt=tt, n=n: c_proj(tt, n)))
                for fn in cwork:
                    fn()

    nc.compile()
    return nc


def _rope_tables():
    inv_freq = 1.0 / (ROPE_BASE ** (np.arange(0, HD, 2, dtype=np.float32) / HD))
    t = np.arange(T, dtype=np.float32)
    freqs = t[:, None] * inv_freq[None, :]
    emb = np.concatenate([freqs, freqs], axis=-1)
    return np.cos(emb).astype(np.float32), np.sin(emb).astype(np.float32)


def _host_prep(x, Wq, Wk, Wv, Wg, Wo, q_gamma, k_gamma, o_gamma):
    x = np.asarray(x, dtype=np.float32)
    Wq = np.asarray(Wq, dtype=np.float32)
    Wk = np.asarray(Wk, dtype=np.float32)
    Wv = np.asarray(Wv, dtype=np.float32)
    Wg = np.asarray(Wg, dtype=np.float32)
    Wo = np.asarray(Wo, dtype=np.float32)
    q_gamma = np.asarray(q_gamma, dtype=np.float32)
    k_gamma = np.asarray(k_gamma, dtype=np.float32)
    o_gamma = np.asarray(o_gamma, dtype=np.float32)

    cos, sin = _rope_tables()

    def perm_tbl(a):
        # (T, HD) -> partition-major [128, TT, HD]
        return np.ascontiguousarray(
            a.reshape(TT, 128, HD).transpose(1, 0, 2)).astype(BF16)

    def perm_kf(a):
        # (H, F) -> partition-major [128, TT, F]
        return np.ascontiguousarray(
            a.reshape(TT, 128, -1).transpose(1, 0, 2)).astype(BF16)

    cosb = perm_tbl(cos)
    sinm_f = np.concatenate([-sin[:, :64], sin[:, 64:]], axis=1)
    sinm = perm_tbl(sinm_f)
    # q_gamma*k_gamma folds into K's private RoPE tables (gamma is applied to
    # K-hat coordinate-wise after the rotation, so scale cos/sin per coord)
    gqk = (q_gamma * k_gamma).astype(np.float32)
    cosk = perm_tbl(cos * gqk[None, :])
    sink = perm_tbl(sinm_f * gqk[None, :])

    xPb = [perm_kf(np.ascontiguousarray(x[b].T)) for b in range(B)]
    per_group = []
    for g in range(4):
        hs = slice(g * F, (g + 1) * F)
        wo_scaled = Wo[:, hs] * np.tile(o_gamma, NHL)[None, :]
        per_group.append({
            "wqP": perm_kf(Wq[hs].T),
            "wkP": perm_kf(Wk[hs].T),
            "wvP": perm_kf(Wv[hs].T),
            "wgP": perm_kf(Wg[hs].T),
            "woP": np.ascontiguousarray(
                wo_scaled.T.reshape(4, 128, H).transpose(1, 0, 2)).astype(BF16),
        })

    in_maps = []
    for c in range(8):
        b, g = c // 4, c % 4
        m = {"xP": xPb[b], "cosb": cosb, "sinm": sinm, "cosk": cosk,
             "sink": sink}
        m.update(per_group[g])
        in_maps.append(m)
    return in_maps


def kernel(x, Wq, Wk, Wv, Wg, Wo, q_gamma, k_gamma, o_gamma):
    if "nc" not in _CACHE:
        _CACHE["nc"] = _build_nc()
    nc = _CACHE["nc"]
    in_maps = _host_prep(x, Wq, Wk, Wv, Wg, Wo, q_gamma, k_gamma, o_gamma)
    res = bass_utils.run_bass_kernel_spmd(nc, in_maps, core_ids=list(range(8)))
    out = np.empty((B, T, H), dtype=np.float32)
    for b in range(B):
        acc = res.results[4 * b]["out"].astype(np.float32)
        for g in range(1, 4):
            acc = acc + res.results[4 * b + g]["out"].astype(np.float32)
        out[b] = acc
    return out



# revision 62
# speedup vs baseline: 1.0269x; 1.0269x over previous
"""Trainium2 Bass kernel for nn_CausalSelfAttention_16149077032974.

Full inputs in, full outputs out. Sharding: data-parallel over B (2 groups of
4 cores), tensor-parallel over heads within a group (4 heads/core). Each core
runs the whole per-head pipeline (QKVG projections, RoPE, QK-RMSNorm, causal
SDPA, output RMSNorm, silu gate, c_proj partial); the c_proj all-reduce is done
on the host while gathering (the partial sums are exact in f32).

Per-core kernel layout choices:
 - All inputs are host-permuted into partition-major DRAM layouts so every
   DMA reads multi-KB contiguous runs per partition line; the startup
   interleaves Wq and x[0:512] in 256KB k-pair pieces on one deep SP queue
   (a single deep queue sustains ~300GB/s; two shallow ones ~100 each) while
   all rope tables ride the ACT queue. ~60 dependency-free dummy matmuls on
   the identity warm the PE's HAM clock gate during the DMA wait.
 - Q/K are projected in (token, feature) layout where RoPE and RMSNorm are
   free-dim ops, then flipped per-head to (head_dim, token) with PE
   transposes. RMS rstd is computed on pre-RoPE values (rotation preserves
   per-token norms) so the normalization fuses into the PSUM evict.
 - Scores are built transposed, S^T[k, q] = K̂·Q̂^T, one 128-row k-tile pair
   per 2-bank PSUM tile; softmax needs no max subtraction (|S|·scale <=
   ~11.3 since q,k are RMS-normalized), so E = exp(scale·S^T) in one wide
   ACTIVATE per pair (diagonal pairs exp the full width; the below-diagonal
   garbage is finite and never read by PV).
 - The softmax denominator is never computed: the output RMSNorm divides by
   rms_d(y_un) directly (the eps·z² correction is <=0.6% of sumsq).
 - SDPA and c_proj run as ONE merged phase: the in-order PE queue is the
   scheduler, weaving score pairs (which gate on exp WAR with only 2 score
   buffers) between PV groups, output transposes and c_proj matmuls so no
   matmul waits at the queue head. ACT keeps a single table set (Exp) for
   the whole phase: 1/sqrt(m) is computed with an exponent-bits initial
   guess + Exp refine + one Newton step on DVE, never touching Sqrt.
 - q_gamma*k_gamma is folded into K̂ via pre-scaled rope tables; o_gamma is
   folded into Wo on the host.
"""

import numpy as np
import ml_dtypes
from contextlib import ExitStack

import concourse.bass as bass
import concourse.tile as tile
from concourse import bacc, mybir
from concourse import bass_utils
from concourse.bass import ts
from concourse.masks import make_identity, make_upper_triangular

BF16 = ml_dtypes.bfloat16
AL = mybir.AluOpType
F32 = mybir.dt.float32
BF = mybir.dt.bfloat16

B, T, H = 2, 2048, 2048
NH, HD = 16, 128
EPS = 1e-5
ROPE_BASE = 10000.0
NHL = 4          # heads per core
F = NHL * HD     # local feature width (512)
TT = T // 128    # 16 token tiles
CH = T // 512    # 4 query chunks
SCALE = 1.0 / float(np.sqrt(HD))

_CACHE = {}


def _build_nc():
    nc = bacc.Bacc("TRN2", target_bir_lowering=False, debug=False)

    # All inputs are pre-permuted on the host into partition-major layouts so
    # every DMA reads long contiguous runs per partition line.
    xT_d = nc.dram_tensor("xP", [128, TT, T], BF, kind="ExternalInput")
    wqT_d = nc.dram_tensor("wqP", [128, TT, F], BF, kind="ExternalInput")
    wkT_d = nc.dram_tensor("wkP", [128, TT, F], BF, kind="ExternalInput")
    wvT_d = nc.dram_tensor("wvP", [128, TT, F], BF, kind="ExternalInput")
    wgT_d = nc.dram_tensor("wgP", [128, TT, F], BF, kind="ExternalInput")
    woT_d = nc.dram_tensor("woP", [128, 4, H], BF, kind="ExternalInput")
    cos_d = nc.dram_tensor("cosb", [128, TT, HD], BF, kind="ExternalInput")
    sin_d = nc.dram_tensor("sinm", [128, TT, HD], BF, kind="ExternalInput")
    cosk_d = nc.dram_tensor("cosk", [128, TT, HD], BF, kind="ExternalInput")
    sink_d = nc.dram_tensor("sink", [128, TT, HD], BF, kind="ExternalInput")
    out_d = nc.dram_tensor("out", [T, H], BF, kind="ExternalOutput")

    with tile.TileContext(nc) as tc:
        with ExitStack() as outer:
            # ---- persistent pools (live across all phases) ----
            consts = outer.enter_context(tc.tile_pool(name="consts", bufs=1))
            qkt = outer.enter_context(tc.tile_pool(name="qkt", bufs=1))
            vpool = outer.enter_context(tc.tile_pool(name="vpool", bufs=1))
            gpool = outer.enter_context(tc.tile_pool(name="gpool", bufs=1))
            ypool = outer.enter_context(tc.tile_pool(name="ypool", bufs=1))
            spool = outer.enter_context(tc.tile_pool(name="spool", bufs=1))

            id128 = consts.tile([128, 128], BF, tag="id")
            make_identity(nc, id128[:, :])
            tri = consts.tile([128, 128], BF, tag="tri")
            make_upper_triangular(nc, tri[:, :], val=1.0, diag=True)
            eps_t = consts.tile([128, 1], F32, tag="eps")
            nc.vector.memset(eps_t[:, :], EPS)

            QT = [qkt.tile([128, T], BF, tag=f"qt{h}", name=f"QT{h}") for h in range(NHL)]
            KT = [qkt.tile([128, T], BF, tag=f"kt{h}", name=f"KT{h}") for h in range(NHL)]
            vaug = vpool.tile([128, TT, NHL, HD], BF, tag="vaug")
            gs = gpool.tile([128, TT, F], BF, tag="gs")       # silu(gate)
            yun = ypool.tile([128, TT, NHL, HD], BF, tag="yun")  # unnormalized y
            m_all = spool.tile([128, TT, NHL], F32, tag="mall")
            stok = spool.tile([128, TT, NHL], F32, tag="stok")

            # ================= phase A: projections =================
            with ExitStack() as pa:
                xpool = pa.enter_context(tc.tile_pool(name="xpool", bufs=1))
                wpool = pa.enter_context(tc.tile_pool(name="wpool", bufs=5))
                pcs = pa.enter_context(tc.tile_pool(name="pcs", bufs=1))
                pstage = pa.enter_context(tc.tile_pool(name="pstage", bufs=2))
                pqr = pa.enter_context(tc.tile_pool(name="pqr", bufs=4))
                pstat = pa.enter_context(tc.tile_pool(name="pstat", bufs=6))
                pqsq = pa.enter_context(tc.tile_pool(name="pqsq", bufs=1))
                psP = pa.enter_context(tc.tile_pool(name="psP", bufs=6, space="PSUM"))
                psTa = pa.enter_context(tc.tile_pool(name="psTa", bufs=2, space="PSUM"))

                xr = xT_d.ap()
                wqr = wqT_d.ap()

                # Interleaved startup on the SP queue in k-pair pieces (a
                # deep single queue sustains ~300GB/s; two shallow queues
                # only got ~100GB/s each): Wq pair then x[tokens 0:512] pair,
                # in warmup consumption order, then the bulk x slabs.
                wq_parts = []
                x0g = []
                for g in range(4):
                    wq_ = wpool.tile([128, 4, F], BF, tag="w", name=f"w_q_{g}")
                    xg = xpool.tile([128, 4, 512], BF, tag=f"x0g{g}", name=f"x0g{g}")
                    wq_parts.append(wq_)
                    x0g.append(xg)
                for g in range(4):
                    for half in range(2):
                        sl = slice(2 * half, 2 * half + 2)
                        kk = 4 * g + 2 * half
                        nc.sync.dma_start(
                            out=wq_parts[g][:, sl, :], in_=wqr[:, kk:kk + 2, :])
                        nc.sync.dma_start(
                            out=x0g[g][:, sl, :], in_=xr[:, kk:kk + 2, 0:512])
                # remaining x, in 512-token slabs (all k-groups each)
                xc = []
                for c in range(1, 4):
                    xt_ = xpool.tile([128, 16, 512], BF, tag=f"xc{c}", name=f"xc{c}")
                    nc.sync.dma_start(
                        out=xt_[:, :, :], in_=xr[:, :, 512 * c:512 * c + 512]
                    )
                    xc.append(xt_)
                # All rope tables go on the ACT queue (idle at startup): the
                # Q tables are needed from the first evicts; K's folded-gamma
                # tables are issued now too, because later the ACT queue is
                # busy with Q-phase evicts and the descriptor generation
                # would only run after those drain (measured +3us PE stall
                # at the Q->K seam otherwise).
                cr = cos_d.ap()
                sr = sin_d.ap()
                ckr = cosk_d.ap()
                skr = sink_d.ap()
                cos0 = pcs.tile([128, 4, HD], BF, tag="cos0")
                sin0 = pcs.tile([128, 4, HD], BF, tag="sin0")
                nc.scalar.dma_start(out=cos0[:, :, :], in_=cr[:, 0:4, :])
                nc.scalar.dma_start(out=sin0[:, :, :], in_=sr[:, 0:4, :])
                cos1 = pcs.tile([128, 12, HD], BF, tag="cos1")
                sin1 = pcs.tile([128, 12, HD], BF, tag="sin1")
                nc.scalar.dma_start(out=cos1[:, :, :], in_=cr[:, 4:16, :])
                nc.scalar.dma_start(out=sin1[:, :, :], in_=sr[:, 4:16, :])
                ck0 = pcs.tile([128, 4, HD], BF, tag="ck0")
                sk0 = pcs.tile([128, 4, HD], BF, tag="sk0")
                nc.scalar.dma_start(out=ck0[:, :, :], in_=ckr[:, 0:4, :])
                nc.scalar.dma_start(out=sk0[:, :, :], in_=skr[:, 0:4, :])
                ck1 = pcs.tile([128, 12, HD], BF, tag="ck1")
                sk1 = pcs.tile([128, 12, HD], BF, tag="sk1")
                nc.scalar.dma_start(out=ck1[:, :, :], in_=ckr[:, 4:16, :])
                nc.scalar.dma_start(out=sk1[:, :, :], in_=skr[:, 4:16, :])
                k_tables = (ck0, sk0, ck1, sk1)

                # Warm the PE's HAM clock gate during the startup DMA wait:
                # ~4us of dependency-free dummy matmuls flip the clock from
                # 1.2GHz to 2.4GHz before the first real projection arrives
                # (the array would otherwise run its first ~3.4us at half
                # rate). id128 is ready immediately; the PE is idle anyway.
                warm_ps = psP.tile([128, F], F32, tag="acc", name="hamwarm")
                for _ in range(60):
                    nc.tensor.matmul(
                        warm_ps[:, 0:128], id128[:, :], id128[:, :],
                        start=True, stop=True,
                    )

                def xap(k, t):
                    # x^T slice (128 h-part, 128 tokens) for k-tile k, t-tile t
                    if t < 4:
                        return x0g[k // 4][:, k % 4, ts(t, 128)]
                    return xc[t // 4 - 1][:, k, ts(t % 4, 128)]

                def cs_ap(t):
                    if t < 4:
                        return cos0[:, t, :], sin0[:, t, :]
                    return cos1[:, t - 4, :], sin1[:, t - 4, :]

                def load_w(w_d):
                    wr = w_d.ap()
                    parts = []
                    for q in range(4):
                        wq_ = wpool.tile([128, 4, F], BF, tag="w", name=f"w_{w_d.name}_{q}")
                        nc.sync.dma_start(out=wq_[:, :, :], in_=wr[:, 4 * q:4 * q + 4, :])
                        parts.append(wq_)
                    return parts

                def matmul_proj(acc, whalves, t):
                    for k in range(TT):
                        wt = whalves[k // 4]
                        nc.tensor.matmul(
                            acc[:, :], xap(k, t), wt[:, k % 4, :],
                            start=(k == 0), stop=(k == TT - 1),
                        )

                # -- Q then K: project + rmsnorm-prescale + rope --
                def qk_phase(targets, use_k_tables, whalves, is_q, warmup):
                    if use_k_tables:
                        ck0, sk0, ck1, sk1 = k_tables

                        def cs_k(t):
                            if t < 4:
                                return ck0[:, t, :], sk0[:, t, :]
                            return ck1[:, t - 4, :], sk1[:, t - 4, :]
                        get_cs = cs_k
                    else:
                        get_cs = cs_ap

                    pend = []

                    def flush(qr_t):
                        # PSUM->SBUF copies split 2 on ACT / 2 on DVE to
                        # balance the two engines' evict-chain load.
                        qr, t = qr_t
                        for hh in range(NHL):
                            tp = psTa.tile([128, 128], BF, tag="tp")
                            nc.tensor.transpose(tp[:, :], qr[:, hh, :], id128[:, :])
                            if hh < 2:
                                nc.scalar.copy(targets[hh][:, ts(t, 128)], tp[:, :])
                            else:
                                nc.vector.tensor_copy(targets[hh][:, ts(t, 128)], tp[:, :])

                    def evict(t, acc):
                        # mean(q^2)+eps per head -> rstd
                        msq = pstat.tile([128, NHL], F32, tag="msq")
                        scr = pqsq.tile([128, F], BF, tag="scr")
                        for hh in range(NHL):
                            nc.scalar.activation(
                                scr[:, ts(hh, 128)], acc[:, ts(hh, 128)],
                                mybir.ActivationFunctionType.Square,
                                accum_out=msq[:, hh:hh + 1],
                            )
                        # ACT sqrt + DVE reciprocal: the DVE-rsqrt variant
                        # was tried and made the warmup-evict DVE burst a
                        # 9us PE stall (4 chains hit DVE at once); the sqrt
                        # table load costs only ~3.6us there.
                        sd = pstat.tile([128, NHL], F32, tag="sd")
                        nc.scalar.activation(
                            sd[:, :], msq[:, :],
                            mybir.ActivationFunctionType.Sqrt,
                            bias=eps_t[:, :], scale=1.0 / HD,
                        )
                        rstd = pstat.tile([128, NHL], F32, tag="rstd")
                        nc.vector.reciprocal(rstd[:, :], sd[:, :])
                        qs = pstage.tile([128, NHL, HD], BF, tag="qs")
                        nc.vector.tensor_mul(
                            qs[:, :, :],
                            acc[:, :].rearrange("p (h d) -> p h d", h=NHL),
                            rstd[:, :, None].broadcast_to([128, NHL, HD]),
                        )
                        cost, sint = get_cs(t)
                        qc = pstage.tile([128, NHL, HD], BF, tag="qc")
                        nc.vector.tensor_mul(
                            qc[:, :, :], qs[:, :, :],
                            cost[:, None, :].broadcast_to([128, NHL, HD]),
                        )
                        rot = pstage.tile([128, NHL, HD], BF, tag="rot")
                        nc.vector.tensor_mul(
                            rot[:, :, 0:64], qs[:, :, 64:128],
                            sint[:, None, 0:64].broadcast_to([128, NHL, 64]),
                        )
                        nc.vector.tensor_mul(
                            rot[:, :, 64:128], qs[:, :, 0:64],
                            sint[:, None, 64:128].broadcast_to([128, NHL, 64]),
                        )
                        qr = pqr.tile([128, NHL, HD], BF, tag="qr")
                        nc.vector.tensor_add(qr[:, :, :], qc[:, :, :], rot[:, :, :])
                        pend.append((qr, t))
                        if len(pend) > 2:
                            flush(pend.pop(0))

                    if warmup:
                        # t=0..3 with 4 concurrently-open accumulators,
                        # consuming k-pairs in DMA arrival order
                        accs = [psP.tile([128, F], F32, tag="acc", name=f"warm{t}")
                                for t in range(4)]
                        for p in range(8):
                            for t in range(4):
                                for k in (2 * p, 2 * p + 1):
                                    nc.tensor.matmul(
                                        accs[t][:, :], xap(k, t),
                                        whalves[k // 4][:, k % 4, :],
                                        start=(k == 0), stop=(k == TT - 1),
                                    )
                        for t in range(4):
                            evict(t, accs[t])
                        t_range = range(4, TT)
                    else:
                        t_range = range(TT)

                    for t in t_range:
                        acc = psP.tile([128, F], F32, tag="acc")
                        matmul_proj(acc, whalves, t)
                        evict(t, acc)
                    while pend:
                        flush(pend.pop(0))

                qk_phase(QT, False, wq_parts, True, True)
                qk_phase(KT, True, load_w(wkT_d), False, False)

                # -- V --
                whalves = load_w(wvT_d)
                for t in range(TT):
                    acc = psP.tile([128, F], F32, tag="acc")
                    matmul_proj(acc, whalves, t)
                    nc.scalar.copy(
                        vaug[:, t, :, :],
                        acc[:, :].rearrange("p (h d) -> p h d", h=NHL),
                    )

                # -- G (silu fused into the evict) --
                whalves = load_w(wgT_d)
                for t in range(TT):
                    acc = psP.tile([128, F], F32, tag="acc")
                    matmul_proj(acc, whalves, t)
                    nc.scalar.activation(
                        gs[:, t, :], acc[:, :], mybir.ActivationFunctionType.Silu
                    )
            # woT is only consumed in phase C; allocating + loading it here
            # (after phase A's pools are released) keeps it out of phase A's
            # SBUF high-water mark and the 2MB transfer hides under phase B.
            wopool = outer.enter_context(tc.tile_pool(name="wopool", bufs=1))
            woT = wopool.tile([128, 4, H], BF, tag="woT")
            nc.sync.dma_start(out=woT[:, :, :], in_=woT_d.ap())

            # ========== phase BC: SDPA with c_proj interleaved ==========
            # Phase B alone is dependency-bound (matmul->exp->PV with 2-3
            # PSUM score buffers leaves the PE idle at unit boundaries) and
            # phase C alone is pure PE work; interleaving C's tiles into the
            # B unit stream fills both engines' stalls.
            with ExitStack() as pb:
                pE = pb.enter_context(tc.tile_pool(name="pE", bufs=21))
                pys = pb.enter_context(tc.tile_pool(name="pys", bufs=4))
                pyscr = pb.enter_context(tc.tile_pool(name="pyscr", bufs=2))
                prs = pb.enter_context(tc.tile_pool(name="prs", bufs=2))
                pyg = pb.enter_context(tc.tile_pool(name="pyg", bufs=10))
                pygT = pb.enter_context(tc.tile_pool(name="pygT", bufs=24))
                pout = pb.enter_context(tc.tile_pool(name="pout", bufs=6))
                psO = pb.enter_context(tc.tile_pool(name="psO", bufs=1, space="PSUM"))
                psTc = pb.enter_context(tc.tile_pool(name="psTc", bufs=1, space="PSUM"))
                pbs = pb.enter_context(ExitStack())
                psS = pbs.enter_context(tc.tile_pool(name="psS", bufs=2, space="PSUM"))
                psY = pbs.enter_context(tc.tile_pool(name="psY", bufs=2, space="PSUM"))
                cpools = {"O": psO, "T": psTc}

                def emit_pair(c, h, elist, kt0):
                    # one S^T pair: two k-tile matmuls + one wide exp.
                    # Diagonal matmuls only compute the causal columns; the
                    # exp still covers the full pair in one ACTIVATE (the
                    # below-diagonal region holds stale PSUM garbage whose
                    # exp is finite and never read by PV).
                    e2 = pE.tile([128, 2, 512], BF, tag="e")
                    s2 = psS.tile([128, 2, 512], F32, tag="s")
                    diag = kt0 >= 4 * c
                    for j in range(2):
                        kt = kt0 + j
                        lo = 0 if kt < 4 * c else 128 * (kt - 4 * c)
                        nc.tensor.matmul(
                            s2[:, j, lo:512], KT[h][:, ts(kt, 128)],
                            QT[h][:, 512 * c + lo:512 * c + 512],
                            start=True, stop=True,
                        )
                    nc.scalar.activation(
                        e2[:, :, :], s2[:, :, :],
                        mybir.ActivationFunctionType.Exp, scale=SCALE,
                    )
                    if diag:
                        for j in range(2):
                            lo = 128 * (kt0 + j - 4 * c)
                            nc.vector.tensor_mul(
                                e2[:, j, lo:lo + 128], e2[:, j, lo:lo + 128],
                                tri[:, :],
                            )
                    elist[kt0 // 2] = e2

                def pv_group(c, h, elist, p):
                    # PV for one psY group (two query tiles); evict is one
                    # CAST, stats are fused square+reduce from SBUF after
                    y_ps = psY.tile([128, 2, HD], F32, tag="y")
                    for qp in range(2):
                        qt = 2 * p + qp
                        t = 4 * c + qt
                        for kt in range(t + 1):
                            nc.tensor.matmul(
                                y_ps[:, qp, :],
                                elist[kt // 2][:, kt % 2, ts(qt, 128)],
                                vaug[:, kt, h, :],
                                start=(kt == 0), stop=(kt == t),
                            )
                    tpair = 4 * c + 2 * p
                    nc.vector.tensor_copy(
                        yun[:, tpair:tpair + 2, h, :], y_ps[:, :, :]
                    )
                    scr = pyscr.tile([128, 2, HD], BF, tag="ysq")
                    nc.vector.tensor_mul(
                        scr[:, :, :], yun[:, tpair:tpair + 2, h, :],
                        yun[:, tpair:tpair + 2, h, :],
                    )
                    ss = pys.tile([128, 2], F32, tag="ss")
                    nc.vector.tensor_reduce(
                        out=ss[:, :], in_=scr[:, :, :],
                        axis=mybir.AxisListType.X, op=mybir.AluOpType.add,
                    )
                    nc.vector.tensor_scalar(
                        out=m_all[:, tpair:tpair + 2, h], in0=ss[:, :],
                        scalar1=1.0 / HD, scalar2=None, op0=AL.mult,
                    )

                def stok_part(c, h_lo, h_hi):
                    # stok = m^-0.5 without touching any new ACT table set:
                    # exponent-bits initial guess (Mitchell), refined through
                    # the already-loaded Exp table, then one Newton step on
                    # DVE (max rel err ~3e-4).
                    msl = m_all[:, 4 * c:4 * c + 4, h_lo:h_hi]
                    shp = [128, 4, h_hi - h_lo]
                    bf_ = prs.tile(shp, F32, tag="rsb")
                    nc.vector.tensor_copy(
                        bf_[:, :, :], msl.bitcast(mybir.dt.int32))
                    l2 = prs.tile(shp, F32, tag="rsl")
                    nc.vector.tensor_scalar(
                        out=l2[:, :, :], in0=bf_[:, :, :],
                        scalar1=2.0 ** -23, scalar2=-(127.0 + 0.043),
                        op0=AL.mult, op1=AL.add,
                    )
                    r0 = prs.tile(shp, F32, tag="rs0")
                    nc.scalar.activation(
                        r0[:, :, :], l2[:, :, :],
                        mybir.ActivationFunctionType.Exp,
                        scale=-0.5 * float(np.log(2.0)),
                    )
                    aa = prs.tile(shp, F32, tag="rsa")
                    nc.vector.tensor_mul(aa[:, :, :], r0[:, :, :], r0[:, :, :])
                    nc.vector.tensor_mul(aa[:, :, :], aa[:, :, :], msl)
                    cc = prs.tile(shp, F32, tag="rsc")
                    nc.vector.tensor_scalar(
                        out=cc[:, :, :], in0=aa[:, :, :],
                        scalar1=-0.5, scalar2=1.5, op0=AL.mult, op1=AL.add,
                    )
                    nc.vector.tensor_mul(
                        stok[:, 4 * c:4 * c + 4, h_lo:h_hi],
                        r0[:, :, :], cc[:, :, :])

                tilesT = {}

                def c_trans(t, hh):
                    # one head's normalize+gate+transpose (all-DVE except the
                    # PSUM evict split); tiny PE item, so psTc gets by with a
                    # single bank when these are woven between heavier items.
                    if t not in tilesT:
                        tilesT[t] = [None] * NHL
                    t1 = pyg.tile([128, HD], BF, tag="t1")
                    nc.vector.tensor_mul(
                        t1[:, :], yun[:, t, hh, :], gs[:, t, ts(hh, 128)]
                    )
                    yg = pyg.tile([128, HD], BF, tag="yg", name=f"yg_{t}_{hh}")
                    nc.vector.tensor_scalar_mul(
                        yg[:, :], t1[:, :], stok[:, t, hh:hh + 1]
                    )
                    tp = cpools["T"].tile([128, 128], BF, tag="tp")
                    nc.tensor.transpose(tp[:, :], yg[:, :], id128[:, :])
                    yt = pygT.tile([128, 128], BF, tag="yt")
                    if hh % 2 == 0:
                        nc.scalar.copy(yt[:, :], tp[:, :])
                    else:
                        nc.vector.tensor_copy(yt[:, :], tp[:, :])
                    tilesT[t][hh] = yt

                def c_proj(t, n):
                    o_ps = cpools["O"].tile([128, 512], F32, tag="o")
                    for f in range(4):
                        nc.tensor.matmul(
                            o_ps[:, :], tilesT[t][f][:, :], woT[:, f, ts(n, 512)],
                            start=(f == 0), stop=(f == 3),
                        )
                    o_sb = pout.tile([128, 512], BF, tag="osb")
                    if n % 2 == 0:
                        nc.vector.tensor_copy(o_sb[:, :], o_ps[:, :])
                    else:
                        nc.scalar.copy(o_sb[:, :], o_ps[:, :])
                    nc.sync.dma_start(
                        out=out_d.ap()[ts(t, 128), ts(n, 512)], in_=o_sb[:, :]
                    )

                # The in-order PE queue is the scheduler: weave the score
                # pairs (which gate on exp WAR with only 2 score buffers)
                # between the PV / transpose / c_proj items so no matmul ever
                # sits at the queue head waiting for the ACT engine.
                units = [(c, h) for c in range(CH) for h in range(NHL)]
                prev = None
                cwork = []   # pending C work items (thunks), in order
                for (c, h) in units:
                    elist = [None] * (2 * c + 2)
                    kt0_order = [4 * c, 4 * c + 2] + list(range(0, 4 * c, 2))
                    sitems = [
                        (lambda kt0=kt0: emit_pair(c, h, elist, kt0))
                        for kt0 in kt0_order
                    ]
                    oitems = []
                    if prev is not None:
                        pc_, ph_, pel = prev
                        oitems.append(lambda: pv_group(pc_, ph_, pel, 0))
                        oitems.append(lambda: pv_group(pc_, ph_, pel, 1))
                        if ph_ == NHL - 1:
                            oitems.append(lambda: stok_part(pc_, 0, NHL))
                            for j in range(4):
                                tt = 4 * pc_ + j
                                for hh in range(NHL):
                                    cwork.append(
                                        (lambda tt=tt, hh=hh: c_trans(tt, hh)))
                                for n in range(4):
                                    cwork.append(
                                        (lambda tt=tt, n=n: c_proj(tt, n)))
                        if (pc_, ph_) == (CH - 1, NHL - 2):
                            # last chunk, second-to-last head done: compute
                            # its stok + transposes for heads 0..2 NOW so the
                            # post-loop tail only owes head 3's serial chain
                            oitems.append(lambda: stok_part(CH - 1, 0, NHL - 1))
                            for j in range(4):
                                tt = 4 * (CH - 1) + j
                                for hh in range(NHL - 1):
                                    oitems.append(
                                        (lambda tt=tt, hh=hh: c_trans(tt, hh)))
                    # take this unit's share of pending C work (8 items per
                    # unit drains a chunk's 32 items over its 4 units)
                    take, cwork = cwork[:8], cwork[8:]
                    oitems += take
                    # weave: two score pairs up front (prime the exp
                    # pipeline), then alternate
                    emit = []
                    si, oi = 0, 0
                    while si < len(sitems) or oi < len(oitems):
                        if si < 2 and si < len(sitems):
                            emit.append(sitems[si]); si += 1
                            continue
                        if oi < len(oitems):
                            emit.append(oitems[oi]); oi += 1
                        if si < len(sitems):
                            emit.append(sitems[si]); si += 1
                    for fn in emit:
                        fn()
                    prev = (c, h, elist)
                # tail: last unit's PV + stok, then release the score/PV
                # PSUM pools so the final chunk's c_proj runs with proper
                # double buffering instead of single-bank serialization
                pc_, ph_, pel = prev
                pv_group(pc_, ph_, pel, 0)
                pv_group(pc_, ph_, pel, 1)
                stok_part(pc_, NHL - 1, NHL)
                pbs.close()
                cpools["O"] = pb.enter_context(
                    tc.tile_pool(name="ptO", bufs=3, space="PSUM"))
                cpools["T"] = pb.enter_context(
                    tc.tile_pool(name="ptT", bufs=2, space="PSUM"))
                for j in range(4):
                    tt = 12 + j
                    cwork.append((lambda tt=tt: c_trans(tt, NHL - 1)))
                    for n in range(4):
                        cwork.append((lambda tt=tt, n=n: c_proj(tt, n)))
                for fn in cwork:
                    fn()

    nc.compile()
    return nc


def _rope_tables():
    inv_freq = 1.0 / (ROPE_BASE ** (np.arange(0, HD, 2, dtype=np.float32) / HD))
    t = np.arange(T, dtype=np.float32)
    freqs = t[:, None] * inv_freq[None, :]
    emb = np.concatenate([freqs, freqs], axis=-1)
    return np.cos(emb).astype(np.float32), np.sin(emb).astype(np.float32)


def _host_prep(x, Wq, Wk, Wv, Wg, Wo, q_gamma, k_gamma, o_gamma):
    x = np.asarray(x, dtype=np.float32)
    Wq = np.asarray(Wq, dtype=np.float32)
    Wk = np.asarray(Wk, dtype=np.float32)
    Wv = np.asarray(Wv, dtype=np.float32)
    Wg = np.asarray(Wg, dtype=np.float32)
    Wo = np.asarray(Wo, dtype=np.float32)
    q_gamma = np.asarray(q_gamma, dtype=np.float32)
    k_gamma = np.asarray(k_gamma, dtype=np.float32)
    o_gamma = np.asarray(o_gamma, dtype=np.float32)

    cos, sin = _rope_tables()

    def perm_tbl(a):
        # (T, HD) -> partition-major [128, TT, HD]
        return np.ascontiguousarray(
            a.reshape(TT, 128, HD).transpose(1, 0, 2)).astype(BF16)

    def perm_kf(a):
        # (H, F) -> partition-major [128, TT, F]
        return np.ascontiguousarray(
            a.reshape(TT, 128, -1).transpose(1, 0, 2)).astype(BF16)

    cosb = perm_tbl(cos)
    sinm_f = np.concatenate([-sin[:, :64], sin[:, 64:]], axis=1)
    sinm = perm_tbl(sinm_f)
    # q_gamma*k_gamma folds into K's private RoPE tables (gamma is applied to
    # K-hat coordinate-wise after the rotation, so scale cos/sin per coord)
    gqk = (q_gamma * k_gamma).astype(np.float32)
    cosk = perm_tbl(cos * gqk[None, :])
    sink = perm_tbl(sinm_f * gqk[None, :])

    xPb = [perm_kf(np.ascontiguousarray(x[b].T)) for b in range(B)]
    per_group = []
    for g in range(4):
        hs = slice(g * F, (g + 1) * F)
        wo_scaled = Wo[:, hs] * np.tile(o_gamma, NHL)[None, :]
        per_group.append({
            "wqP": perm_kf(Wq[hs].T),
            "wkP": perm_kf(Wk[hs].T),
            "wvP": perm_kf(Wv[hs].T),
            "wgP": perm_kf(Wg[hs].T),
            "woP": np.ascontiguousarray(
                wo_scaled.T.reshape(4, 128, H).transpose(1, 0, 2)).astype(BF16),
        })

    in_maps = []
    for c in range(8):
        b, g = c // 4, c % 4
        m = {"xP": xPb[b], "cosb": cosb, "sinm": sinm, "cosk": cosk,
             "sink": sink}
        m.update(per_group[g])
        in_maps.append(m)
    return in_maps


def kernel(x, Wq, Wk, Wv, Wg, Wo, q_gamma, k_gamma, o_gamma):
    if "nc" not in _CACHE:
        _CACHE["nc"] = _build_nc()
    nc = _CACHE["nc"]
    in_maps = _host_prep(x, Wq, Wk, Wv, Wg, Wo, q_gamma, k_gamma, o_gamma)
    res = bass_utils.run_bass_kernel_spmd(nc, in_maps, core_ids=list(range(8)))
    out = np.empty((B, T, H), dtype=np.float32)
    for b in range(B):
        acc = res.results[4 * b]["out"].astype(np.float32)
        for g in range(1, 4):
            acc = acc + res.results[4 * b + g]["out"].astype(np.float32)
        out[b] = acc
    return out

